# revision 11
# baseline (speedup 1.0000x reference)
"""TRN2 Bass kernel: MultiHeadSelfAttention (B=4, S=2048, D=1024, H=16, DK=64).

Sharding: 8 cores = 4 batches x 2 head-groups (8 heads each).

Key optimizations over the v1 kernel:
- Token compaction: the reference multiplies the output by the padding mask
  and masked keys get softmax weight exactly 0 (exp(-1e6-max) underflows), so
  attention only involves the unmasked tokens. The host gathers those (~1024
  of 2048) and pads to SP (multiple of 128); outputs are scattered back.
- fp16 operands everywhere (10-bit mantissa ~= TF32): 1 cyc/row matmuls at
  any width, half the SBUF/DMA of f32.
- P-stationary PV: stationary P^T chunk [keys x q], moving [V_h | 1] -> O in
  [q x dh] layout with the softmax denominator as column 64. Normalization is
  a per-partition tensor_scalar_mul fused into the PSUM eviction (gpsimd).
- One full-row max (DVE) + one full-row exp (Act) per (head, q-tile), reading
  a multi-bank PSUM tile.
- Software-pipelined phase 2 with a lag between the QK/softmax stream and the
  PV/out-projection stream.
"""

import numpy as np

B, S, D, H, DK = 4, 2048, 1024, 16, 64
HG = 2            # head groups (tensor-parallel)
HL = H // HG      # heads per core = 8
DH = HL * DK      # 512 per-core head width
KT = D // 128     # 8 contraction tiles
SP_DEFAULT = 1152

_cache = {}


def _build(SP):
    from concourse import bacc
    import concourse.mybir as mybir
    import concourse.tile as tile

    f32 = mybir.dt.float32
    f16 = mybir.dt.float16
    Exp = mybir.ActivationFunctionType.Exp
    AX = mybir.AxisListType.X
    NT = SP // 128

    nc = bacc.Bacc("TRN2", target_bir_lowering=False, debug=False, num_devices=8)

    xT_d = nc.dram_tensor("xT", [D, SP], f16, kind="ExternalInput")
    wq_d = nc.dram_tensor("wq", [D, DH], f16, kind="ExternalInput")
    wk_d = nc.dram_tensor("wk", [D, DH], f16, kind="ExternalInput")
    wv_d = nc.dram_tensor("wv", [D, DH], f16, kind="ExternalInput")
    wo_d = nc.dram_tensor("wo", [DH, D], f16, kind="ExternalInput")
    y_d = nc.dram_tensor("y", [SP, D], f16, kind="ExternalOutput")

    with tile.TileContext(nc) as tc:
        with (
            tc.tile_pool(name="persist", bufs=1) as pp,
            tc.tile_pool(name="psS", bufs=2, space="PSUM") as psS,
            tc.tile_pool(name="psO", bufs=1, space="PSUM") as psO,
            tc.tile_pool(name="psY", bufs=1, space="PSUM") as psY,
        ):
            qT = pp.tile([128, 4, SP], f16, tag="qT")
            kT = pp.tile([128, 4, SP], f16, tag="kT")
            # V with a ones column per head: blocks of 66 = [V_h(64) | 1 | pad]
            v2 = pp.tile([128, NT, HL, 66], f16, tag="v2")
            nc.gpsimd.memset(v2[:, :, :, 64:65], 1.0)
            wor = pp.tile([128, 4, D], f16, tag="wor")
            nc.gpsimd.dma_start(wor[:], wo_d.rearrange("(c p) n -> p c n", p=128))

            # ---- phase 1: projections ----
            xr = pp.tile([128, KT, SP], f16, tag="xr")
            nc.gpsimd.dma_start(xr[:], xT_d.rearrange("(t p) s -> p t s", p=128))
            wvr = pp.tile([128, KT, DH], f16, tag="wvr")
            nc.gpsimd.dma_start(wvr[:], wv_d.rearrange("(t p) n -> p t n", p=128))
            wkr = pp.tile([128, KT, DH], f16, tag="wkr")
            nc.gpsimd.dma_start(wkr[:], wk_d.rearrange("(t p) n -> p t n", p=128))
            wqr = pp.tile([128, KT, DH], f16, tag="wqr")
            nc.gpsimd.dma_start(wqr[:], wq_d.rearrange("(t p) n -> p t n", p=128))

            # column chunks of a row of SP scores / tokens (PSUM banks)
            chunks = []
            c0 = 0
            while c0 < SP:
                c1 = min(c0 + 512, SP)
                chunks.append((c0, c1))
                c0 = c1

            for wi, (wr, dst) in enumerate(((wkr, kT), (wqr, qT))):
                for p in range(4):
                    ps = psS.tile([128, 3, 512], f32, tag="mmS")
                    for (c0, c1) in chunks:
                        bank, off = c0 // 512, c0 % 512
                        for k in range(KT):
                            nc.tensor.matmul(
                                ps[:, bank, off:off + (c1 - c0)],
                                wr[:, k, p * 128:(p + 1) * 128],
                                xr[:, k, c0:c1],
                                start=(k == 0),
                                stop=(k == KT - 1),
                            )
                    sflat = ps[:].rearrange("p a b -> p (a b)")[:, 0:SP]
                    if (wi * 4 + p) % 2 == 0:
                        nc.vector.tensor_copy(dst[:, p, :], sflat)
                    else:
                        nc.scalar.copy(dst[:, p, :], sflat)
            for sc in range(NT):
                if sc % 2 == 0:
                    psv = psO.tile([128, 512], f32, tag="mmO")
                else:
                    psv = psY.tile([128, 512], f32, tag="mmY")
                for k in range(KT):
                    nc.tensor.matmul(
                        psv[:],
                        xr[:, k, sc * 128:(sc + 1) * 128],
                        wvr[:, k, :],
                        start=(k == 0),
                        stop=(k == KT - 1),
                    )
                nc.gpsimd.tensor_copy(
                    v2[:, sc, :, 0:64],
                    psv[:].rearrange("p (h w) -> p h w", w=64),
                )

            # ---- phase 2: attention + output projection (software pipelined)
            with (
                tc.tile_pool(name="pexp", bufs=3) as pexp,
                tc.tile_pool(name="ptbp", bufs=2) as ptbp,
                tc.tile_pool(name="stats", bufs=6) as st,
                tc.tile_pool(name="osbp", bufs=2) as osbp,
                tc.tile_pool(name="oTp", bufs=2) as oTp,
                tc.tile_pool(name="yp", bufs=2) as yp,
            ):
                sched = [(i, h) for i in range(NT) for h in range(HL)]
                LAG = 3     # QK/softmax stream leads the PV stream
                OLAG = 2    # out-projection lags the last PV of its q-tile
                state = {}

                def issue_qk(i, h):
                    p, r0 = h // 2, (h % 2) * 64
                    ps = psS.tile([128, 3, 512], f32, tag="mmS")
                    for (c0, c1) in chunks:
                        bank, off = c0 // 512, c0 % 512
                        nc.tensor.matmul(
                            ps[:, bank, off:off + (c1 - c0)],
                            qT[r0:r0 + DK, p, i * 128:(i + 1) * 128],
                            kT[r0:r0 + DK, p, c0:c1],
                            start=True,
                            stop=True,
                        )
                    sflat = ps[:].rearrange("p a b -> p (a b)")[:, 0:SP]
                    nm = st.tile([128, 1], f32, tag="nm")
                    nc.vector.tensor_reduce(
                        nm[:], sflat, axis=AX, op=mybir.AluOpType.max, negate=True,
                    )
                    p_sb = pexp.tile([128, SP], f16, tag="p")
                    nc.scalar.activation(p_sb[:], sflat, Exp, bias=nm[:], scale=1.0)
                    ptb = ptbp.tile([128, NT, 128], f16, tag="ptb")
                    nc.sync.dma_start(ptb[:], p_sb[:], transpose=True)
                    state[(i, h)] = ptb

                def issue_pv(i, h):
                    ptb = state.pop((i, h))
                    if h == 0:
                        osb_t = osbp.tile([128, HL, 64], f16, tag="osb")
                        state[("osb", i)] = osb_t
                    o_sb = state[("osb", i)]
                    o_ps = psO.tile([128, 512], f32, tag="mmO")
                    for kc in range(NT):
                        nc.tensor.matmul(
                            o_ps[:, 0:65],
                            ptb[:, kc, :],
                            v2[:, kc, h, 0:65],
                            start=(kc == 0),
                            stop=(kc == NT - 1),
                        )
                    ot = st.tile([128, 65], f32, tag="ot")
                    nc.gpsimd.tensor_copy(ot[:], o_ps[:, 0:65])
                    nc.gpsimd.normalize_recip(o_sb[:, h, :], ot[:, 0:64], ot[:, 64:65])

                def issue_oproj(i):
                    o_sb = state.pop(("osb", i))
                    oT = oTp.tile([128, 4, 128], f16, tag="oT")
                    nc.sync.dma_start(
                        oT[:], o_sb[:].rearrange("p a b -> p (a b)"), transpose=True)
                    y_sb = yp.tile([128, D], f16, tag="y")
                    for half in range(2):
                        yq = psY.tile([128, 512], f32, tag="mmY")
                        for c in range(4):
                            nc.tensor.matmul(
                                yq[:],
                                oT[:, c, :],
                                wor[:, c, half * 512:(half + 1) * 512],
                                start=(c == 0),
                                stop=(c == 3),
                            )
                        nc.gpsimd.tensor_copy(y_sb[:, half * 512:(half + 1) * 512], yq[:])
                    nc.sync.dma_start(y_d[i * 128:(i + 1) * 128, :], y_sb[:])

                n = len(sched)
                for idx in range(n + LAG + OLAG):
                    if idx < n:
                        issue_qk(*sched[idx])
                    j = idx - LAG
                    if 0 <= j < n:
                        issue_pv(*sched[j])
                    k2 = idx - LAG - OLAG
                    if 0 <= k2 < n and sched[k2][1] == HL - 1:
                        issue_oproj(sched[k2][0])

    nc.compile()
    return nc


def _prep_inputs(x, mask, WQ, WK, WV, WO, SP):
    idxs = [np.nonzero(mask[b])[0] for b in range(B)]
    in_maps = []
    for c in range(8):
        b, g = c // 2, c % 2
        idx = idxs[b]
        perm = np.array(
            [dk * H + (g * HL + hh) for hh in range(HL) for dk in range(DK)]
        )
        xT = np.zeros((D, SP), np.float16)
        xT[:, :len(idx)] = x[b][idx].T
        in_maps.append({
            "xT": xT,
            "wq": np.ascontiguousarray(WQ[:, perm] / np.sqrt(DK)).astype(np.float16),
            "wk": np.ascontiguousarray(WK[:, perm]).astype(np.float16),
            "wv": np.ascontiguousarray(WV[:, perm]).astype(np.float16),
            "wo": np.ascontiguousarray(WO[g * DH:(g + 1) * DH, :]).astype(np.float16),
        })
    return in_maps, idxs


def kernel(x, mask, WQ, WK, WV, WO, _want_results=False, _trace=False):
    from concourse.bass_utils import run_bass_kernel_spmd

    x = np.asarray(x)
    mask = np.asarray(mask)
    nb_max = int(mask.sum(axis=1).max())
    SP = max(SP_DEFAULT, -(-nb_max // 128) * 128)
    if ("nc", SP) not in _cache:
        _cache[("nc", SP)] = _build(SP)
    nc = _cache[("nc", SP)]
    _cache["nc"] = nc  # convenience alias for external tooling
    in_maps, idxs = _prep_inputs(x, mask, np.asarray(WQ, np.float32),
                                 np.asarray(WK, np.float32),
                                 np.asarray(WV, np.float32),
                                 np.asarray(WO, np.float32), SP)
    res = run_bass_kernel_spmd(nc, in_maps, list(range(8)), trace=_trace)
    out = np.zeros((B, S, D), np.float32)
    for b in range(B):
        idx = idxs[b]
        yb = (res.results[2 * b]["y"].astype(np.float32)
              + res.results[2 * b + 1]["y"].astype(np.float32))
        out[b][idx] = np.abs(yb[:len(idx)])
    if _want_results:
        return out, res
    return out


# revision 14
# speedup vs baseline: 1.0403x; 1.0403x over previous
"""TRN2 Bass kernel: MultiHeadSelfAttention (B=4, S=2048, D=1024, H=16, DK=64).

Sharding: 8 cores = 4 batches x 2 head-groups (8 heads each).

Key optimizations over the v1 kernel:
- Token compaction: the reference multiplies the output by the padding mask
  and masked keys get softmax weight exactly 0 (exp(-1e6-max) underflows), so
  attention only involves the unmasked tokens. The host gathers those (~1024
  of 2048) and pads to SP (multiple of 128); outputs are scattered back.
- fp16 operands everywhere (10-bit mantissa ~= TF32): 1 cyc/row matmuls at
  any width, half the SBUF/DMA of f32.
- P-stationary PV: stationary P^T chunk [keys x q], moving [V_h | 1] -> O in
  [q x dh] layout with the softmax denominator as column 64. Normalization is
  a per-partition tensor_scalar_mul fused into the PSUM eviction (gpsimd).
- One full-row max (DVE) + one full-row exp (Act) per (head, q-tile), reading
  a multi-bank PSUM tile.
- Software-pipelined phase 2 with a lag between the QK/softmax stream and the
  PV/out-projection stream.
"""

import numpy as np

B, S, D, H, DK = 4, 2048, 1024, 16, 64
HG = 2            # head groups (tensor-parallel)
HL = H // HG      # heads per core = 8
DH = HL * DK      # 512 per-core head width
KT = D // 128     # 8 contraction tiles
SP_DEFAULT = 1152

_cache = {}


def _build(SP):
    from concourse import bacc
    import concourse.mybir as mybir
    import concourse.tile as tile

    f32 = mybir.dt.float32
    f16 = mybir.dt.float16
    Exp = mybir.ActivationFunctionType.Exp
    AX = mybir.AxisListType.X
    NT = SP // 128

    nc = bacc.Bacc("TRN2", target_bir_lowering=False, debug=False, num_devices=8)

    xT_d = nc.dram_tensor("xT", [D, SP], f16, kind="ExternalInput")
    wq_d = nc.dram_tensor("wq", [D, DH], f16, kind="ExternalInput")
    wk_d = nc.dram_tensor("wk", [D, DH], f16, kind="ExternalInput")
    wv_d = nc.dram_tensor("wv", [D, DH], f16, kind="ExternalInput")
    wo_d = nc.dram_tensor("wo", [DH, D], f16, kind="ExternalInput")
    y_d = nc.dram_tensor("y", [SP, D], f16, kind="ExternalOutput")

    with tile.TileContext(nc) as tc:
        with (
            tc.tile_pool(name="persist", bufs=1) as pp,
            tc.tile_pool(name="psS", bufs=2, space="PSUM") as psS,
            tc.tile_pool(name="psO", bufs=1, space="PSUM") as psO,
            tc.tile_pool(name="psY", bufs=1, space="PSUM") as psY,
        ):
            qT = pp.tile([128, 4, SP], f16, tag="qT")
            kT = pp.tile([128, 4, SP], f16, tag="kT")
            # V with a ones column per head: blocks of 66 = [V_h(64) | 1 | pad]
            v2 = pp.tile([128, NT, HL, 66], f16, tag="v2")
            nc.gpsimd.memset(v2[:, :, :, 64:65], 1.0)
            wor = pp.tile([128, 4, D], f16, tag="wor")
            nc.gpsimd.dma_start(wor[:], wo_d.rearrange("(c p) n -> p c n", p=128))

            # ---- phase 1: projections ----
            xr = pp.tile([128, KT, SP], f16, tag="xr")
            nc.gpsimd.dma_start(xr[:], xT_d.rearrange("(t p) s -> p t s", p=128))
            wvr = pp.tile([128, KT, DH], f16, tag="wvr")
            nc.gpsimd.dma_start(wvr[:], wv_d.rearrange("(t p) n -> p t n", p=128))
            wkr = pp.tile([128, KT, DH], f16, tag="wkr")
            nc.gpsimd.dma_start(wkr[:], wk_d.rearrange("(t p) n -> p t n", p=128))
            wqr = pp.tile([128, KT, DH], f16, tag="wqr")
            nc.gpsimd.dma_start(wqr[:], wq_d.rearrange("(t p) n -> p t n", p=128))

            # column chunks of a row of SP scores / tokens (PSUM banks)
            chunks = []
            c0 = 0
            while c0 < SP:
                c1 = min(c0 + 512, SP)
                chunks.append((c0, c1))
                c0 = c1

            for wi, (wr, dst) in enumerate(((wkr, kT), (wqr, qT))):
                for p in range(4):
                    ps = psS.tile([128, 3, 512], f32, tag="mmS")
                    for (c0, c1) in chunks:
                        bank, off = c0 // 512, c0 % 512
                        for k in range(KT):
                            nc.tensor.matmul(
                                ps[:, bank, off:off + (c1 - c0)],
                                wr[:, k, p * 128:(p + 1) * 128],
                                xr[:, k, c0:c1],
                                start=(k == 0),
                                stop=(k == KT - 1),
                            )
                    sflat = ps[:].rearrange("p a b -> p (a b)")[:, 0:SP]
                    if (wi * 4 + p) % 2 == 0:
                        nc.vector.tensor_copy(dst[:, p, :], sflat)
                    else:
                        nc.scalar.copy(dst[:, p, :], sflat)
            for sc in range(NT):
                if sc % 2 == 0:
                    psv = psO.tile([128, 512], f32, tag="mmO")
                else:
                    psv = psY.tile([128, 512], f32, tag="mmY")
                for k in range(KT):
                    nc.tensor.matmul(
                        psv[:],
                        xr[:, k, sc * 128:(sc + 1) * 128],
                        wvr[:, k, :],
                        start=(k == 0),
                        stop=(k == KT - 1),
                    )
                nc.gpsimd.tensor_copy(
                    v2[:, sc, :, 0:64],
                    psv[:].rearrange("p (h w) -> p h w", w=64),
                )

            # ---- phase 2: attention + output projection (software pipelined)
            import os as _os
            with (
                tc.tile_pool(name="pexp", bufs=int(_os.environ.get("PEXP", "3"))) as pexp,
                tc.tile_pool(name="ptbp", bufs=int(_os.environ.get("PTB", "2"))) as ptbp,
                tc.tile_pool(name="stats", bufs=6) as st,
                tc.tile_pool(name="osbp", bufs=2) as osbp,
                tc.tile_pool(name="oTp", bufs=2) as oTp,
                tc.tile_pool(name="yp", bufs=2) as yp,
            ):
                import os
                sched = [(i, h) for i in range(NT) for h in range(HL)]
                LAG = int(os.environ.get("LAG", "3"))
                OLAG = int(os.environ.get("OLAG", "2"))
                PVFIRST = int(os.environ.get("PVFIRST", "0"))
                state = {}

                def issue_qk(i, h):
                    p, r0 = h // 2, (h % 2) * 64
                    ps = psS.tile([128, 3, 512], f32, tag="mmS")
                    for (c0, c1) in chunks:
                        bank, off = c0 // 512, c0 % 512
                        nc.tensor.matmul(
                            ps[:, bank, off:off + (c1 - c0)],
                            qT[r0:r0 + DK, p, i * 128:(i + 1) * 128],
                            kT[r0:r0 + DK, p, c0:c1],
                            start=True,
                            stop=True,
                        )
                    sflat = ps[:].rearrange("p a b -> p (a b)")[:, 0:SP]
                    nm = st.tile([128, 1], f32, tag="nm")
                    nc.vector.tensor_reduce(
                        nm[:], sflat, axis=AX, op=mybir.AluOpType.max, negate=True,
                    )
                    p_sb = pexp.tile([128, SP], f16, tag="p")
                    nc.scalar.activation(p_sb[:], sflat, Exp, bias=nm[:], scale=1.0)
                    ptb = ptbp.tile([128, NT, 128], f16, tag="ptb")
                    nc.sync.dma_start(ptb[:], p_sb[:], transpose=True)
                    state[(i, h)] = ptb

                def issue_pv(i, h):
                    ptb = state.pop((i, h))
                    if h == 0:
                        osb_t = osbp.tile([128, HL, 64], f16, tag="osb")
                        state[("osb", i)] = osb_t
                    o_sb = state[("osb", i)]
                    o_ps = psO.tile([128, 512], f32, tag="mmO")
                    for kc in range(NT):
                        nc.tensor.matmul(
                            o_ps[:, 0:65],
                            ptb[:, kc, :],
                            v2[:, kc, h, 0:65],
                            start=(kc == 0),
                            stop=(kc == NT - 1),
                        )
                    ot = st.tile([128, 65], f32, tag="ot")
                    nc.gpsimd.tensor_copy(ot[:], o_ps[:, 0:65])
                    nc.gpsimd.normalize_recip(o_sb[:, h, :], ot[:, 0:64], ot[:, 64:65])

                def issue_oproj(i):
                    o_sb = state.pop(("osb", i))
                    oT = oTp.tile([128, 4, 128], f16, tag="oT")
                    nc.sync.dma_start(
                        oT[:], o_sb[:].rearrange("p a b -> p (a b)"), transpose=True)
                    y_sb = yp.tile([128, D], f16, tag="y")
                    for half in range(2):
                        yq = psY.tile([128, 512], f32, tag="mmY")
                        for c in range(4):
                            nc.tensor.matmul(
                                yq[:],
                                oT[:, c, :],
                                wor[:, c, half * 512:(half + 1) * 512],
                                start=(c == 0),
                                stop=(c == 3),
                            )
                        nc.gpsimd.tensor_copy(y_sb[:, half * 512:(half + 1) * 512], yq[:])
                    nc.sync.dma_start(y_d[i * 128:(i + 1) * 128, :], y_sb[:])

                n = len(sched)
                for idx in range(n + LAG + OLAG):
                    j = idx - LAG
                    if PVFIRST and 0 <= j < n:
                        issue_pv(*sched[j])
                    if idx < n:
                        issue_qk(*sched[idx])
                    if not PVFIRST and 0 <= j < n:
                        issue_pv(*sched[j])
                    k2 = idx - LAG - OLAG
                    if 0 <= k2 < n and sched[k2][1] == HL - 1:
                        issue_oproj(sched[k2][0])

    nc.compile()
    return nc


def _prep_inputs(x, mask, WQ, WK, WV, WO, SP):
    idxs = [np.nonzero(mask[b])[0] for b in range(B)]
    in_maps = []
    for c in range(8):
        b, g = c // 2, c % 2
        idx = idxs[b]
        perm = np.array(
            [dk * H + (g * HL + hh) for hh in range(HL) for dk in range(DK)]
        )
        xT = np.zeros((D, SP), np.float16)
        xT[:, :len(idx)] = x[b][idx].T
        in_maps.append({
            "xT": xT,
            "wq": np.ascontiguousarray(WQ[:, perm] / np.sqrt(DK)).astype(np.float16),
            "wk": np.ascontiguousarray(WK[:, perm]).astype(np.float16),
            "wv": np.ascontiguousarray(WV[:, perm]).astype(np.float16),
            "wo": np.ascontiguousarray(WO[g * DH:(g + 1) * DH, :]).astype(np.float16),
        })
    return in_maps, idxs


def kernel(x, mask, WQ, WK, WV, WO, _want_results=False, _trace=False):
    from concourse.bass_utils import run_bass_kernel_spmd

    x = np.asarray(x)
    mask = np.asarray(mask)
    nb_max = int(mask.sum(axis=1).max())
    SP = max(SP_DEFAULT, -(-nb_max // 128) * 128)
    if ("nc", SP) not in _cache:
        _cache[("nc", SP)] = _build(SP)
    nc = _cache[("nc", SP)]
    _cache["nc"] = nc  # convenience alias for external tooling
    in_maps, idxs = _prep_inputs(x, mask, np.asarray(WQ, np.float32),
                                 np.asarray(WK, np.float32),
                                 np.asarray(WV, np.float32),
                                 np.asarray(WO, np.float32), SP)
    res = run_bass_kernel_spmd(nc, in_maps, list(range(8)), trace=_trace)
    out = np.zeros((B, S, D), np.float32)
    for b in range(B):
        idx = idxs[b]
        yb = (res.results[2 * b]["y"].astype(np.float32)
              + res.results[2 * b + 1]["y"].astype(np.float32))
        out[b][idx] = np.abs(yb[:len(idx)])
    if _want_results:
        return out, res
    return out
